# revision 1
# baseline (speedup 1.0000x reference)
"""Causal self-attention (B=2, T=2048, C=1024, H=16) on 8 TRN2 NeuronCores.

Sharding: tensor-parallel over heads (2 heads/core) for QKV projection and
attention; AllToAll converts the head-sharded attention output into a
sequence-sharded layout; each core then computes its 512-row slice of the
output projection. Host only slices/casts inputs and concatenates outputs.

Device math in bf16 with fp32 PSUM accumulation:
  - x is pre-transposed on host to xT [C, B*T] (bf16) so every matmul
    contraction has channels on the partition axis.
  - Scores are built transposed (S^T [keys, queries]) so softmax
    normalization sums arrive for free from a ones-augmented P^T @ [V|1]
    matmul, and no on-device transposes are needed anywhere.
  - exp on ScalarE (fp32-accurate LUT); no max-subtraction needed since
    scores are O(+-8).
"""
import os
import math
import threading

import numpy as np
import ml_dtypes

import concourse.bass as bass
import concourse.tile as tile
from concourse import mybir, bacc, bass_utils

B, T, C, H = 2, 2048, 1024, 16
D = C // H                 # 64
NCORES = 8
HPC = H // NCORES          # heads per core = 2
HC = HPC * D               # head-channels per core = 128
BT = B * T                 # 4096
TQ = 512                   # query chunk
TKT = 128                  # key tile
ROWS = BT // NCORES        # output rows per core = 512
SM_SCALE = 1.0 / math.sqrt(D)

F32 = mybir.dt.float32
BF16 = mybir.dt.bfloat16
BF16_NP = ml_dtypes.bfloat16


def _build_program():
    nc = bacc.Bacc("TRN2", target_bir_lowering=False, debug=False,
                   num_devices=NCORES)
    xt = nc.dram_tensor("xt", [C, BT], BF16, kind="ExternalInput").ap()
    wqkv = nc.dram_tensor("wqkv", [C, 3 * HC], BF16, kind="ExternalInput").ap()
    wproj = nc.dram_tensor("wproj", [C, C], BF16, kind="ExternalInput").ap()
    bq = nc.dram_tensor("bq", [HC, 1], F32, kind="ExternalInput").ap()
    bk = nc.dram_tensor("bk", [HC, 1], F32, kind="ExternalInput").ap()
    bv = nc.dram_tensor("bv", [1, HC], BF16, kind="ExternalInput").ap()
    bproj = nc.dram_tensor("bproj", [1, C], BF16, kind="ExternalInput").ap()
    masks = nc.dram_tensor("masks", [TQ // TKT, TKT, TQ], BF16,
                           kind="ExternalInput").ap()
    outp = nc.dram_tensor("out", [ROWS, C], F32, kind="ExternalOutput").ap()

    KT = C // 128          # 8 contraction tiles over channels
    NCH = BT // TQ         # 8 T-chunks over B*T
    SPC = TQ // D          # 8 strips of 64 rows per chunk (one per core)

    with tile.TileContext(nc) as tc:
        with (
            tc.tile_pool(name="consts", bufs=1) as consts,
            tc.tile_pool(name="xpool", bufs=2) as xpool,
            tc.tile_pool(name="ppool", bufs=6) as ppool,
            tc.tile_pool(name="npool", bufs=2) as npool,
            tc.tile_pool(name="opool", bufs=2) as opool,
            tc.tile_pool(name="ps_o", bufs=2, space="PSUM") as ps_o,
            tc.tile_pool(name="dram", bufs=1, space="DRAM") as dram,
        ):
            # ---- stage 0: weights & constants ----
            wqkv_sb = []
            for kt in range(KT):
                w1 = consts.tile([128, 3 * HC], BF16, name=f"wqkv_sb{kt}")
                nc.sync.dma_start(out=w1, in_=wqkv[128 * kt:128 * (kt + 1), :])
                wqkv_sb.append(w1)
            # big weights not needed until ~100us in: keep them off the SP
            # HWDGE queue so the stage-1 stream starts immediately
            wproj_sb = []
            for kt in range(KT):
                w2 = consts.tile([128, C], BF16, name=f"wproj_sb{kt}")
                nc.gpsimd.dma_start(out=w2, in_=wproj[128 * kt:128 * (kt + 1), :])
                wproj_sb.append(w2)
            ones_sb = consts.tile([1, 128], BF16, name="ones_sb")
            nc.vector.memset(ones_sb, 1.0)
            bq_sb = consts.tile([HC, 1], F32, name="bq_sb")
            nc.sync.dma_start(out=bq_sb, in_=bq)
            bk_sb = consts.tile([HC, 1], F32, name="bk_sb")
            nc.sync.dma_start(out=bk_sb, in_=bk)
            bv_sb = consts.tile([1, HC], BF16, name="bv_sb")
            nc.sync.dma_start(out=bv_sb, in_=bv)
            bproj_sb = consts.tile([1, C], BF16, name="bproj_sb")
            nc.sync.dma_start(out=bproj_sb, in_=bproj)
            masks_sb = consts.tile([TKT, TQ // TKT, TQ], BF16, name="masks_sb")
            nc.gpsimd.dma_start(out=masks_sb, in_=masks.rearrange("r p q -> p r q"))

            qT_b = [consts.tile([HC, T], BF16, name=f"qT_sb{b}")
                    for b in range(B)]
            kT_b = [consts.tile([HC, T], BF16, name=f"kT_sb{b}")
                    for b in range(B)]
            v_sb = [consts.tile([128, 2 * (D + 1)], BF16, name=f"v_sb{tt}")
                    for tt in range(BT // 128)]

            # per-chunk exchange buffers: block s of chunk c = queries
            # [64s, 64s+64) of that chunk, owned by core s
            a2a_in = [dram.tile([NCORES, HC, D], BF16, name=f"a2a_in{c}")
                      for c in range(NCH)]
            a2a_out = [dram.tile([NCORES, HC, D], BF16, name=f"a2a_out{c}")
                       for c in range(NCH)]

            def stage4_pair(cA, cB):
                """Output projection for two 64-row strips, column-packed."""
                ylhs = {}
                for ci, c in enumerate((cA, cB)):
                    yy = opool.tile([128, SPC, D], BF16, tag=f"ylhs{ci}",
                                    name=f"ylhs{ci}")
                    nc.sync.dma_start(
                        out=yy, in_=a2a_out[c].rearrange("k p q -> p k q"))
                    ylhs[c] = yy
                for n in range(C // TQ):
                    po = ps_o.tile([128, TQ], F32, tag="po", name="po")
                    for ci, c in enumerate((cA, cB)):
                        pslice = po[D * ci:D * (ci + 1), :]
                        for kt in range(KT):
                            nc.tensor.matmul(
                                pslice,
                                lhsT=ylhs[c][:, kt, :],
                                rhs=wproj_sb[kt][:, TQ * n:TQ * (n + 1)],
                                start=(kt == 0), stop=False)
                        nc.tensor.matmul(
                            pslice, lhsT=ones_sb[:, 0:D],
                            rhs=bproj_sb[:, TQ * n:TQ * (n + 1)],
                            start=False, stop=True)
                    osb = opool.tile([128, TQ], F32, tag="osb", name="osb")
                    nc.vector.tensor_copy(out=osb, in_=po)
                    for ci, c in enumerate((cA, cB)):
                        nc.sync.dma_start(
                            out=outp[D * c:D * (c + 1), TQ * n:TQ * (n + 1)],
                            in_=osb[D * ci:D * (ci + 1), :])

            done_chunks = []
            for b in range(B):
                # ---- stage 1: QKV projection for this batch ----
                with (
                    tc.tile_pool(name=f"ps_qk{b}", bufs=3, space="PSUM") as ps_qk,
                    tc.tile_pool(name=f"ps_v{b}", bufs=2, space="PSUM") as ps_v,
                ):
                    for cl in range(NCH // B):
                        c = (NCH // B) * b + cl
                        xt_t = []
                        for kt in range(KT):
                            xx = xpool.tile([128, TQ], BF16, tag=f"xt{kt}")
                            nc.sync.dma_start(
                                out=xx,
                                in_=xt[128 * kt:128 * (kt + 1),
                                       TQ * c:TQ * (c + 1)])
                            xt_t.append(xx)
                        for which, off, bias, scale in (
                            ("q", 0, bq_sb, 1.0),
                            ("k", HC, bk_sb, SM_SCALE),
                        ):
                            ps = ps_qk.tile([HC, TQ], F32, tag="qk")
                            for kt in range(KT):
                                nc.tensor.matmul(
                                    ps,
                                    lhsT=wqkv_sb[kt][:, off:off + HC],
                                    rhs=xt_t[kt],
                                    start=(kt == 0), stop=(kt == KT - 1))
                            dst = qT_b[b] if which == "q" else kT_b[b]
                            nc.scalar.activation(
                                out=dst[:, TQ * cl:TQ * (cl + 1)], in_=ps,
                                func=mybir.ActivationFunctionType.Identity,
                                bias=bias, scale=scale)
                        # V (natural layout, ones-augmented)
                        for s in range(TQ // 128):
                            tt = 4 * c + s
                            ps = ps_v.tile([128, HC], F32, tag="v")
                            for kt in range(KT):
                                nc.tensor.matmul(
                                    ps,
                                    lhsT=xt_t[kt][:, 128 * s:128 * (s + 1)],
                                    rhs=wqkv_sb[kt][:, 2 * HC:3 * HC],
                                    start=(kt == 0), stop=False)
                            nc.tensor.matmul(ps, lhsT=ones_sb, rhs=bv_sb,
                                             start=False, stop=True)
                            vt = v_sb[tt]
                            nc.vector.tensor_copy(out=vt[:, 0:D], in_=ps[:, 0:D])
                            nc.vector.tensor_copy(out=vt[:, D + 1:2 * D + 1],
                                                  in_=ps[:, D:2 * D])
                            nc.vector.memset(vt[:, D:D + 1], 1.0)
                            nc.vector.memset(vt[:, 2 * D + 1:2 * D + 2], 1.0)

                # ---- stage 2: attention for this batch, largest chunks
                # first; each chunk's exchange + output projection follows
                # immediately and hides under later chunks' attention ----
                with (
                    tc.tile_pool(name=f"ps_s{b}", bufs=4, space="PSUM") as ps_s,
                    tc.tile_pool(name=f"ps_y{b}", bufs=1, space="PSUM") as ps_y,
                ):
                    for jl in reversed(range(T // TQ)):
                        cidx = (T // TQ) * b + jl
                        q0 = TQ * jl
                        nkt = (TQ // TKT) * (jl + 1)
                        y_ps = [ps_y.tile([D + 1, TQ], F32, tag=f"y{h}",
                                          name=f"y_ps{h}")
                                for h in range(HPC)]
                        pts = []
                        for kt in range(nkt):
                            k0 = TKT * kt
                            r = kt - (TQ // TKT) * jl
                            pt_pair = []
                            for h in range(HPC):
                                hp = D * h
                                ss = ps_s.tile([TKT, TQ], F32, tag="s",
                                               name=f"ss{h}")
                                nc.tensor.matmul(
                                    ss,
                                    lhsT=kT_b[b][hp:hp + D, k0:k0 + TKT],
                                    rhs=qT_b[b][hp:hp + D, q0:q0 + TQ],
                                    start=True, stop=True)
                                pt = ppool.tile([TKT, TQ], BF16, tag=f"pt{h}",
                                                name=f"pt{h}")
                                nc.scalar.activation(
                                    out=pt, in_=ss,
                                    func=mybir.ActivationFunctionType.Exp)
                                if r >= 0:
                                    nc.vector.tensor_mul(pt, pt,
                                                         masks_sb[:, r, :])
                                pt_pair.append(pt)
                            pts.append(pt_pair)
                        for kt in range(nkt):
                            vt = v_sb[(T // 128) * b + kt]
                            for h in range(HPC):
                                nc.tensor.matmul(
                                    y_ps[h],
                                    lhsT=vt[:, (D + 1) * h:(D + 1) * (h + 1)],
                                    rhs=pts[kt][h],
                                    start=(kt == 0), stop=(kt == nkt - 1))
                        for h in range(HPC):
                            recip = npool.tile([1, TQ], F32, tag="recip")
                            nc.vector.reciprocal(recip, y_ps[h][D:D + 1, :])
                            recip_b = npool.tile([D, TQ], F32, tag="recipb")
                            nc.gpsimd.partition_broadcast(recip_b, recip)
                            yt = npool.tile([D, TQ], BF16, tag="yt")
                            nc.vector.tensor_mul(yt, y_ps[h][0:D, :], recip_b)
                            nc.sync.dma_start(
                                out=a2a_in[cidx][:, D * h:D * (h + 1), :]
                                    .rearrange("s p q -> p s q"),
                                in_=yt.rearrange("p (s q) -> p s q", s=SPC))
                        nc.gpsimd.collective_compute(
                            "AllToAll", mybir.AluOpType.bypass,
                            replica_groups=[list(range(NCORES))],
                            ins=[a2a_in[cidx].opt()],
                            outs=[a2a_out[cidx].opt()])
                        done_chunks.append(cidx)
                        if len(done_chunks) % 2 == 0:
                            stage4_pair(done_chunks[-2], done_chunks[-1])

    nc.compile()
    return nc


_lock = threading.Lock()
_cached_nc = None
last_results = None  # BassKernelResults of the most recent kernel() call


def _get_program():
    global _cached_nc
    with _lock:
        if _cached_nc is None:
            _cached_nc = _build_program()
    return _cached_nc


def _host_inputs(x, W_qkv, b_qkv, W_proj, b_proj):
    bf = lambda a: np.ascontiguousarray(a).astype(BF16_NP)
    x = np.asarray(x, dtype=np.float32)
    W_qkv = np.asarray(W_qkv, dtype=np.float32)
    b_qkv = np.asarray(b_qkv, dtype=np.float32)
    W_proj = np.asarray(W_proj, dtype=np.float32)
    b_proj = np.asarray(b_proj, dtype=np.float32)

    xt = bf(x.reshape(BT, C).T)                     # [C, BT]
    wproj = bf(W_proj)                              # [C, C]
    bproj = bf(b_proj.reshape(1, C))
    r = np.arange(TQ // TKT)[:, None, None]
    k = np.arange(TKT)[None, :, None]
    q = np.arange(TQ)[None, None, :]
    masks = ((k + TKT * r) <= q).astype(BF16_NP)    # [4, 128, 512]

    in_maps = []
    for i in range(NCORES):
        sel = slice(HC * i, HC * (i + 1))
        wq = W_qkv[:, sel]
        wk = W_qkv[:, C + HC * i:C + HC * (i + 1)]
        wv = W_qkv[:, 2 * C + HC * i:2 * C + HC * (i + 1)]
        in_maps.append({
            "xt": xt,
            "wqkv": bf(np.concatenate([wq, wk, wv], axis=1)),
            "wproj": wproj,
            "bq": np.ascontiguousarray(
                b_qkv[sel].reshape(HC, 1)).astype(np.float32),
            "bk": np.ascontiguousarray(
                (b_qkv[C + HC * i:C + HC * (i + 1)] * SM_SCALE)
                .reshape(HC, 1)).astype(np.float32),
            "bv": b_qkv[2 * C + HC * i:2 * C + HC * (i + 1)]
                .reshape(1, HC).astype(BF16_NP),
            "bproj": bproj,
            "masks": masks,
        })
    return in_maps


def kernel(x, W_qkv, b_qkv, W_proj, b_proj):
    global last_results
    nc = _get_program()
    in_maps = _host_inputs(x, W_qkv, b_qkv, W_proj, b_proj)
    trace = bool(int(os.environ.get("KERNEL_TRACE", "0")))
    res = bass_utils.run_bass_kernel_spmd(
        nc, in_maps, core_ids=list(range(NCORES)), trace=trace)
    last_results = res
    # core s's output rows are strip s (64 rows) of every 512-row chunk
    arr = np.stack([res.results[s]["out"].reshape(BT // TQ, D, C)
                    for s in range(NCORES)], axis=1)   # [chunk, core, 64, C]
    return np.ascontiguousarray(arr.reshape(B, T, C))



# revision 10
# speedup vs baseline: 1.0797x; 1.0797x over previous
"""Causal self-attention (B=2, T=2048, C=1024, H=16) on 8 TRN2 NeuronCores.

Sharding: tensor-parallel over heads (2 heads/core) for QKV projection and
attention; AllToAll (per pair of 512-token chunks) converts the head-sharded
attention output into a sequence-sharded layout (128 contiguous tokens per
core per pair); each core then computes the output projection for its strips.

Key structure (v2):
  - x pre-transposed on host to xT [C, B*T] bf16; all projection matmuls
    contract channels on the partition axis.
  - Scores built transposed (S^T [keys, queries]) with K=64 per head; the
    two heads' score matmuls are issued adjacently at high priority so they
    run CONCURRENTLY in different PE row-groups (tile_position (0,0)/(64,0)).
  - Diagonal 128-key tiles only compute the causally-live query range
    (N = 512-128d); a single shared [128,128] staircase mask zeroes the
    in-tile triangle.
  - PV is done queries-on-partitions: y[q,65] += P^T[:,qb]^T @ [V|1], so the
    softmax denominator lands per-partition; normalize = [128,1] reciprocal
    + tensor_scalar multiply (no partition broadcasts, no [1,512] recips).
  - Stage 4 transposes the gathered y rows back to [ch, tok] with 8 PE
    transposes per pair, then runs the output projection with M=128.
"""
import os
import math
import threading

import numpy as np
import ml_dtypes

import concourse.bass as bass
import concourse.tile as tile
from concourse import mybir, bacc, bass_utils

B, T, C, H = 2, 2048, 1024, 16
D = C // H                 # 64
NCORES = 8
HPC = H // NCORES          # heads per core = 2
HC = HPC * D               # head-channels per core = 128
BT = B * T                 # 4096
TQ = 512                   # query chunk
TKT = 128                  # key tile
NCH = BT // TQ             # 8 chunks over B*T
SM_SCALE = 1.0 / math.sqrt(D)
NPAIR = 4                  # 4 pairs of chunks (1024 tokens each)

F32 = mybir.dt.float32
BF16 = mybir.dt.bfloat16
BF16_NP = ml_dtypes.bfloat16


def _build_program():
    nc = bacc.Bacc("TRN2", target_bir_lowering=False, debug=False,
                   num_devices=NCORES)
    xt = nc.dram_tensor("xt", [C, BT], BF16, kind="ExternalInput").ap()
    wqkv = nc.dram_tensor("wqkv", [C, 3 * HC], BF16, kind="ExternalInput").ap()
    wproj = nc.dram_tensor("wproj", [C, C], BF16, kind="ExternalInput").ap()
    bq = nc.dram_tensor("bq", [HC, 1], F32, kind="ExternalInput").ap()
    bk = nc.dram_tensor("bk", [HC, 1], F32, kind="ExternalInput").ap()
    bv = nc.dram_tensor("bv", [1, HC], BF16, kind="ExternalInput").ap()
    bproj = nc.dram_tensor("bproj", [1, C], BF16, kind="ExternalInput").ap()
    stair = nc.dram_tensor("stair", [TKT, TKT], BF16,
                           kind="ExternalInput").ap()
    ident = nc.dram_tensor("ident", [128, 128], BF16,
                           kind="ExternalInput").ap()
    outp = nc.dram_tensor("out", [NPAIR * 128, C], F32,
                          kind="ExternalOutput").ap()

    KT = C // 128          # 8 contraction tiles over channels

    with tile.TileContext(nc) as tc:
        with (
            tc.tile_pool(name="consts", bufs=1) as consts,
            tc.tile_pool(name="xpool", bufs=2) as xpool,
            tc.tile_pool(name="ppool", bufs=1) as ppool,
            tc.tile_pool(name="npool", bufs=3) as npool,
            tc.tile_pool(name="opool", bufs=2) as opool,
            tc.tile_pool(name="ps_o", bufs=1, space="PSUM") as ps_o,
            tc.tile_pool(name="ps_qk", bufs=1, space="PSUM") as ps_qk,
            tc.tile_pool(name="ps_v", bufs=1, space="PSUM") as ps_v,
            tc.tile_pool(name="ps_s", bufs=1, space="PSUM") as ps_s,
            tc.tile_pool(name="ps_y", bufs=2, space="PSUM") as ps_y,
            tc.tile_pool(name="dram", bufs=1, space="DRAM") as dram,
        ):
            # ---- stage 0: weights & constants ----
            # first-chunk x tiles go out first so stage 1 starts ASAP
            xt_pre = []
            for kt in range(KT):
                xx = xpool.tile([128, TQ], BF16, tag=f"xt{kt}")
                nc.sync.dma_start(out=xx, in_=xt[128 * kt:128 * (kt + 1), 0:TQ])
                xt_pre.append(xx)
            wqkv_sb = []
            for kt in range(KT):
                w1 = consts.tile([128, 3 * HC], BF16, name=f"wqkv_sb{kt}")
                eng = (nc.sync, nc.scalar)[kt % 2]
                eng.dma_start(out=w1, in_=wqkv[128 * kt:128 * (kt + 1), :])
                wqkv_sb.append(w1)
            ones_sb = consts.tile([1, 128], BF16, name="ones_sb")
            nc.vector.memset(ones_sb, 1.0)
            bq_sb = consts.tile([HC, 1], F32, name="bq_sb")
            nc.scalar.dma_start(out=bq_sb, in_=bq)
            bk_sb = consts.tile([HC, 1], F32, name="bk_sb")
            nc.scalar.dma_start(out=bk_sb, in_=bk)
            bv_sb = consts.tile([1, HC], BF16, name="bv_sb")
            nc.scalar.dma_start(out=bv_sb, in_=bv)
            bproj_sb = consts.tile([1, C], BF16, name="bproj_sb")
            nc.scalar.dma_start(out=bproj_sb, in_=bproj)
            stair_sb = consts.tile([TKT, TKT], BF16, name="stair_sb")
            nc.scalar.dma_start(out=stair_sb, in_=stair)
            ident_sb = consts.tile([128, 128], BF16, name="ident_sb")
            nc.scalar.dma_start(out=ident_sb, in_=ident)
            # big out-proj weights not needed until late: keep them off the
            # hot queues
            wproj_sb = []
            for kt in range(KT):
                w2 = consts.tile([128, C], BF16, name=f"wproj_sb{kt}")
                nc.gpsimd.dma_start(out=w2, in_=wproj[128 * kt:128 * (kt + 1), :])
                wproj_sb.append(w2)

            qT_b = [consts.tile([HC, T], BF16, name=f"qT_sb{b}")
                    for b in range(B)]
            kT_b = [consts.tile([HC, T], BF16, name=f"kT_sb{b}")
                    for b in range(B)]
            # V natural layout, per 128-token strip: [v_h0 | 1 | v_h1 | 1]
            v_sb = [consts.tile([128, 2 * (D + 1)], BF16, name=f"v_sb{tt}")
                    for tt in range(BT // 128)]

            # per-pair exchange buffers: block s = tokens [128s,128s+128) of
            # the pair's 1024 tokens, all my head-channels
            a2a_in = [dram.tile([NCORES, 128, HC], BF16, name=f"a2a_in{p}")
                      for p in range(NPAIR)]
            a2a_out = [dram.tile([NCORES, 128, HC], BF16, name=f"a2a_out{p}")
                       for p in range(NPAIR)]

            def stage4(p):
                """Output projection for pair p's 128-token strip."""
                yin = opool.tile([128, C], BF16, tag="yin", name="yin")
                for s in range(NCORES):
                    nc.sync.dma_start(
                        out=yin[:, 128 * s:128 * (s + 1)], in_=a2a_out[p][s])
                tp = ps_o.tile([128, C], BF16, tag="tp", name="tp")
                for s in range(KT):
                    nc.tensor.transpose(tp[:, 128 * s:128 * (s + 1)],
                                        yin[:, 128 * s:128 * (s + 1)],
                                        ident_sb)
                yT = opool.tile([128, C], BF16, tag="yT", name="yT")
                nc.vector.tensor_copy(out=yT, in_=tp)
                for n in range(C // TQ):
                    po = ps_o.tile([128, TQ], F32, tag="po", name="po")
                    for kt in range(KT):
                        nc.tensor.matmul(
                            po, lhsT=yT[:, 128 * kt:128 * (kt + 1)],
                            rhs=wproj_sb[kt][:, TQ * n:TQ * (n + 1)],
                            start=(kt == 0), stop=False)
                    nc.tensor.matmul(
                        po, lhsT=ones_sb,
                        rhs=bproj_sb[:, TQ * n:TQ * (n + 1)],
                        start=False, stop=True)
                    osb = opool.tile([128, TQ], F32, tag="osb", name="osb")
                    nc.vector.tensor_copy(out=osb, in_=po)
                    nc.sync.dma_start(
                        out=outp[128 * p:128 * (p + 1), TQ * n:TQ * (n + 1)],
                        in_=osb)

            for b in range(B):
                # ---- stage 1: QKV projection for this batch ----
                for cl in range(NCH // B):
                    c = (NCH // B) * b + cl
                    if c == 0:
                        xt_t = xt_pre
                    else:
                        xt_t = []
                        for kt in range(KT):
                            xx = xpool.tile([128, TQ], BF16, tag=f"xt{kt}")
                            nc.sync.dma_start(
                                out=xx,
                                in_=xt[128 * kt:128 * (kt + 1),
                                       TQ * c:TQ * (c + 1)])
                            xt_t.append(xx)
                    for which, off, bias, scale in (
                        ("q", 0, bq_sb, 1.0),
                        ("k", HC, bk_sb, SM_SCALE),
                    ):
                        ps = ps_qk.tile([HC, TQ], F32, tag="qk")
                        for kt in range(KT):
                            nc.tensor.matmul(
                                ps,
                                lhsT=wqkv_sb[kt][:, off:off + HC],
                                rhs=xt_t[kt],
                                start=(kt == 0), stop=(kt == KT - 1))
                        dst = qT_b[b] if which == "q" else kT_b[b]
                        nc.vector.tensor_scalar(
                            out=dst[:, TQ * cl:TQ * (cl + 1)], in0=ps,
                            scalar1=scale, scalar2=bias,
                            op0=mybir.AluOpType.mult,
                            op1=mybir.AluOpType.add)
                    # V (natural layout, ones-augmented)
                    for s in range(TQ // 128):
                        tt = 4 * c + s
                        ps = ps_v.tile([128, HC], F32, tag="v")
                        for kt in range(KT):
                            nc.tensor.matmul(
                                ps,
                                lhsT=xt_t[kt][:, 128 * s:128 * (s + 1)],
                                rhs=wqkv_sb[kt][:, 2 * HC:3 * HC],
                                start=(kt == 0), stop=False)
                        nc.tensor.matmul(ps, lhsT=ones_sb[:, 0:HC],
                                         rhs=bv_sb, start=False, stop=True)
                        vt = v_sb[tt]
                        nc.vector.tensor_copy(out=vt[:, 0:D], in_=ps[:, 0:D])
                        nc.vector.tensor_copy(out=vt[:, D + 1:2 * D + 1],
                                              in_=ps[:, D:2 * D])
                        nc.vector.memset(vt[:, D:D + 1], 1.0)
                        nc.vector.memset(vt[:, 2 * D + 1:2 * D + 2], 1.0)

                # ---- stage 2: attention, largest chunks first; each pair's
                # exchange + output projection follows immediately ----
                for jl in reversed(range(T // TQ)):
                    nkt = 4 * jl + 4
                    q0g = TQ * jl
                    pts = {0: [], 1: []}
                    for kb in range(nkt):
                        d = kb - 4 * jl
                        q0 = 128 * max(d, 0)
                        ss_pair = []
                        with tc.high_priority():
                            for h in range(HPC):
                                hp = D * h
                                ss = ps_s.tile([TKT, TQ], F32,
                                               tag=f"ss{h}", name=f"ss{h}")
                                nc.tensor.matmul(
                                    ss[:, q0:TQ],
                                    lhsT=kT_b[b][hp:hp + D,
                                                 128 * kb:128 * (kb + 1)],
                                    rhs=qT_b[b][hp:hp + D,
                                                q0g + q0:q0g + TQ],
                                    start=True, stop=True)
                                ss_pair.append(ss)
                        for h in range(HPC):
                            pt = ppool.tile([TKT, TQ], BF16,
                                            tag=f"pt{h}_{kb}",
                                            name=f"pt{h}_{kb}")
                            nc.scalar.activation(
                                out=pt[:, q0:TQ], in_=ss_pair[h][:, q0:TQ],
                                func=mybir.ActivationFunctionType.Exp)
                            if d >= 0:
                                eng = nc.vector if h == 0 else nc.gpsimd
                                eng.tensor_mul(pt[:, q0:q0 + 128],
                                               pt[:, q0:q0 + 128],
                                               stair_sb)
                            pts[h].append(pt)
                    # PV: queries on partitions, [V|1] as rhs
                    p = 2 * b + (0 if jl >= 2 else 1)
                    idx = 0 if jl in (3, 1) else 1
                    for qb in range(TQ // 128):
                        yp = ps_y.tile([128, 2 * (D + 1)], F32,
                                       tag="y", name=f"y{qb}")
                        for h in range(HPC):
                            ys = yp[:, (D + 1) * h:(D + 1) * (h + 1)]
                            nky = 4 * jl + qb + 1
                            for kb in range(nky):
                                vt = v_sb[16 * b + kb]
                                nc.tensor.matmul(
                                    ys,
                                    lhsT=pts[h][kb][:, 128 * qb:
                                                    128 * (qb + 1)],
                                    rhs=vt[:, (D + 1) * h:
                                           (D + 1) * (h + 1)],
                                    start=(kb == 0), stop=(kb == nky - 1))
                            recip = npool.tile([128, 1], F32, tag="recip")
                            nc.vector.reciprocal(recip, ys[:, D:D + 1])
                            yt = npool.tile([128, D], BF16, tag="yt")
                            nc.vector.tensor_scalar_mul(
                                yt, ys[:, 0:D], recip)
                            nc.gpsimd.dma_start(
                                out=a2a_in[p][4 * idx + qb, :,
                                              D * h:D * (h + 1)],
                                in_=yt)
                    if idx == 1:
                        nc.gpsimd.collective_compute(
                            "AllToAll", mybir.AluOpType.bypass,
                            replica_groups=[list(range(NCORES))],
                            ins=[a2a_in[p].opt()],
                            outs=[a2a_out[p].opt()])
                        stage4(p)

    nc.compile()
    return nc


_lock = threading.Lock()
_cached_nc = None
last_results = None  # BassKernelResults of the most recent kernel() call


def _get_program():
    global _cached_nc
    with _lock:
        if _cached_nc is None:
            _cached_nc = _build_program()
    return _cached_nc


def _host_inputs(x, W_qkv, b_qkv, W_proj, b_proj):
    bf = lambda a: np.ascontiguousarray(a).astype(BF16_NP)
    x = np.asarray(x, dtype=np.float32)
    W_qkv = np.asarray(W_qkv, dtype=np.float32)
    b_qkv = np.asarray(b_qkv, dtype=np.float32)
    W_proj = np.asarray(W_proj, dtype=np.float32)
    b_proj = np.asarray(b_proj, dtype=np.float32)

    xt = bf(x.reshape(BT, C).T)                     # [C, BT]
    wproj = bf(W_proj)                              # [C, C]
    bproj = bf(b_proj.reshape(1, C))
    k = np.arange(TKT)[:, None]
    j = np.arange(TKT)[None, :]
    stair = (k <= j).astype(BF16_NP)                # [128, 128]
    identm = np.eye(128, dtype=BF16_NP)

    in_maps = []
    for i in range(NCORES):
        sel = slice(HC * i, HC * (i + 1))
        wq = W_qkv[:, sel]
        wk = W_qkv[:, C + HC * i:C + HC * (i + 1)]
        wv = W_qkv[:, 2 * C + HC * i:2 * C + HC * (i + 1)]
        in_maps.append({
            "xt": xt,
            "wqkv": bf(np.concatenate([wq, wk, wv], axis=1)),
            "wproj": wproj,
            "bq": np.ascontiguousarray(
                b_qkv[sel].reshape(HC, 1)).astype(np.float32),
            "bk": np.ascontiguousarray(
                (b_qkv[C + HC * i:C + HC * (i + 1)] * SM_SCALE)
                .reshape(HC, 1)).astype(np.float32),
            "bv": b_qkv[2 * C + HC * i:2 * C + HC * (i + 1)]
                .reshape(1, HC).astype(BF16_NP),
            "bproj": bproj,
            "stair": stair,
            "ident": identm,
        })
    return in_maps


# pair p covers chunks (cA, cB); pair tokens 0-511 = chunk cA, 512-1023 = cB
_PAIR_CHUNKS = [(3, 2), (1, 0), (7, 6), (5, 4)]


def kernel(x, W_qkv, b_qkv, W_proj, b_proj):
    global last_results
    nc = _get_program()
    in_maps = _host_inputs(x, W_qkv, b_qkv, W_proj, b_proj)
    trace = bool(int(os.environ.get("KERNEL_TRACE", "0")))
    res = bass_utils.run_bass_kernel_spmd(
        nc, in_maps, core_ids=list(range(NCORES)), trace=trace)
    last_results = res
    out = np.empty((BT, C), dtype=np.float32)
    for s in range(NCORES):
        o = res.results[s]["out"]            # [4*128, C]
        for p, (cA, cB) in enumerate(_PAIR_CHUNKS):
            strip = o[128 * p:128 * (p + 1)]
            if s < 4:
                t0 = TQ * cA + 128 * s
            else:
                t0 = TQ * cB + 128 * (s - 4)
            out[t0:t0 + 128] = strip
    return np.ascontiguousarray(out.reshape(B, T, C))
